# revision 3
# baseline (speedup 1.0000x reference)
"""BigBird sparse-attention head (B=8, T=2048, C=1024, H=64) on 8 TRN2 cores.

Sharding: data-parallel over batch — core b computes batch element b.
Per-core pipeline (all in one Tile kernel, scores kept in transposed
"sT" layout [k_t partitions, q_t free] so softmax normalization folds
into the P@V matmul via a ones-column on v):

  1. x [2048,1024] f32 --cast-DMA--> bf16, PE-transpose -> xT [1024,2048]
  2. Wq/Wk/Wv --cast--> bf16, PE-transpose; Wq/Wk packed as one lhsT
  3. qT,kT [64,2048] = WqkT.T @ xT   (PSUM f32, copied to bf16 SBUF)
     v    [2048,64] = xT.T @ WvT, stored as v_ext [128,16,65] with a
     ones column (col 64) so P@V also produces row sums
  4. for each q-block i (512 wide), k-chunk j (128): sT = kT_j.T @ qT_i;
     pT = exp(0.125*sT) (ACT, bf16); pT *= maskT tile (host-built fp8
     bigbird mask, cast to bf16 during DMA); out[128,65] += pT_q.T @ v_ext_j
  5. out[:, :64] * 1/out[:,64] -> DRAM

The bigbird mask (causal & (local|global|random)) depends only on
random_cols; it is built host-side once as fp8 (exact 0.0/1.0), shared
by all cores, and applied multiplicatively after exp. exp needs no
max-subtraction: scores are ~N(0,1) (|s|<~8) so exp is safe in f32/bf16.
"""

from contextlib import ExitStack

import numpy as np

B, T, C, H = 8, 2048, 1024, 64
WINDOW, N_GLOBAL, N_RANDOM = 64, 64, 64
NCORES = 8
TCH = T // 128   # 16 token chunks
CCH = C // 128   # 8 embed chunks
QB = 512         # q-block width
NQB = T // QB    # 4 q-blocks

_CACHE: dict = {}


def _build_module():
    import concourse.bacc as bacc
    import concourse.mybir as mybir
    import concourse.tile as tile

    F32 = mybir.dt.float32
    FP8 = mybir.dt.float8e4

    nc = bacc.Bacc(
        "TRN2", target_bir_lowering=False, debug=False, enable_asserts=False
    )
    x_d = nc.dram_tensor("x", [T, C], F32, kind="ExternalInput").ap()
    wq_d = nc.dram_tensor("wq", [H, C], F32, kind="ExternalInput").ap()
    wk_d = nc.dram_tensor("wk", [H, C], F32, kind="ExternalInput").ap()
    wv_d = nc.dram_tensor("wv", [H, C], F32, kind="ExternalInput").ap()
    m_d = nc.dram_tensor("maskt", [T, T], FP8, kind="ExternalInput").ap()
    o_d = nc.dram_tensor("out", [T, H], F32, kind="ExternalOutput").ap()

    with tile.TileContext(nc) as tc:
        _emit(tc, o_d, x_d, wq_d, wk_d, wv_d, m_d)
    nc.compile()
    return nc


def _emit(tc, o_d, x_d, wq_d, wk_d, wv_d, m_d):
    import concourse.mybir as mybir
    from concourse.bass import ts
    from concourse.masks import make_identity

    nc = tc.nc
    F32 = mybir.dt.float32
    BF16 = mybir.dt.bfloat16
    EXP = mybir.ActivationFunctionType.Exp

    with ExitStack() as ctx:
        sb = ctx.enter_context(tc.tile_pool(name="sb", bufs=3))
        persist = ctx.enter_context(tc.tile_pool(name="persist", bufs=1))
        ppool = ctx.enter_context(tc.tile_pool(name="ppool", bufs=3))
        mpool = ctx.enter_context(tc.tile_pool(name="mpool", bufs=2))
        opool = ctx.enter_context(tc.tile_pool(name="opool", bufs=2))
        ps_tp = ctx.enter_context(tc.tile_pool(name="ps_tp", bufs=2, space="PSUM"))
        ps_big = ctx.enter_context(tc.tile_pool(name="ps_big", bufs=2, space="PSUM"))
        ps_o = ctx.enter_context(tc.tile_pool(name="ps_o", bufs=1, space="PSUM"))

        ident = persist.tile([128, 128], BF16, tag="ident", name="ident")
        make_identity(nc, ident[:])

        # ---- x: cast-load + PE-transpose into xT chunks ----------------
        xt = persist.tile([128, CCH, T], BF16, tag="xt", name="xt")
        for t in range(TCH):
            xs = sb.tile([128, C], BF16, tag="xstage", name="xs")
            nc.gpsimd.dma_start(xs[:], x_d[ts(t, 128), :])  # f32 -> bf16 cast
            for cg in range(CCH // 4):
                pt4 = ps_tp.tile([128, 512], BF16, tag="tp", name="pt4")
                for k in range(4):
                    c = cg * 4 + k
                    nc.tensor.transpose(
                        pt4[:, ts(k, 128)], xs[:, ts(c, 128)], ident[:]
                    )
                # one strided copy drops the 4 transposed blocks into xt
                dst = xt[:, cg * 4 : cg * 4 + 4, ts(t, 128)]
                if cg % 2 == 0:
                    nc.vector.tensor_copy(dst, pt4.rearrange("p (c f) -> p c f", f=128))
                else:
                    nc.scalar.copy(dst, pt4.rearrange("p (c f) -> p c f", f=128))

        # ---- weights: cast-load, PE-transpose, pack q|k ------------------
        wq_s = sb.tile([H, C], BF16, tag="wstage", name="wq_s")
        nc.gpsimd.dma_start(wq_s[:], wq_d[:])
        wk_s = sb.tile([H, C], BF16, tag="wstage", name="wk_s")
        nc.gpsimd.dma_start(wk_s[:], wk_d[:])
        wv_s = sb.tile([H, C], BF16, tag="wstage", name="wv_s")
        nc.gpsimd.dma_start(wv_s[:], wv_d[:])

        wqk = persist.tile([128, CCH, 128], BF16, tag="wqk", name="wqk")
        wvt = persist.tile([128, CCH, H], BF16, tag="wvt", name="wvt")
        for c in range(CCH):
            pw = ps_tp.tile([128, 512], BF16, tag="tp", name="pw")
            nc.tensor.transpose(pw[:, 0:64], wq_s[:, ts(c, 128)], ident[0:64, 0:64])
            nc.tensor.transpose(pw[:, 64:128], wk_s[:, ts(c, 128)], ident[0:64, 0:64])
            nc.tensor.transpose(pw[:, 128:192], wv_s[:, ts(c, 128)], ident[0:64, 0:64])
            nc.vector.tensor_copy(wqk[:, c, :], pw[:, 0:128])
            nc.vector.tensor_copy(wvt[:, c, :], pw[:, 128:192])

        # ---- projections qT, kT [64, 2048] -------------------------------
        qt_s = persist.tile([64, T], BF16, tag="qt", name="qt_s")
        kt_s = persist.tile([64, T], BF16, tag="kt", name="kt_s")
        for tb in range(NQB):
            pq = ps_big.tile([128, 512], F32, tag="big", name="pq")
            for c in range(CCH):
                nc.tensor.matmul(
                    pq[:],
                    wqk[:, c, :],
                    xt[:, c, ts(tb, 512)],
                    start=(c == 0),
                    stop=(c == CCH - 1),
                )
            nc.scalar.copy(qt_s[:, ts(tb, 512)], pq[0:64, :])
            nc.vector.tensor_copy(kt_s[:, ts(tb, 512)], pq[64:128, :])

        # ---- v rows + ones column: v_ext [128, 16, 65] -------------------
        vext = persist.tile([128, TCH, H + 1], BF16, tag="vext", name="vext")
        for t in range(TCH):
            pv = ps_tp.tile([128, 512], F32, tag="tp", name="pv")
            for c in range(CCH):
                nc.tensor.matmul(
                    pv[:, 0:64],
                    xt[:, c, ts(t, 128)],
                    wvt[:, c, :],
                    start=(c == 0),
                    stop=(c == CCH - 1),
                )
            nc.scalar.copy(vext[:, t, 0:64], pv[:, 0:64])
        nc.gpsimd.memset(vext[:, :, 64:65], 1.0)

        # ---- attention ---------------------------------------------------
        m_r = m_d.rearrange("(j p) q -> p j q", p=128)
        for i in range(NQB):
            nj = 4 * i + 4
            osums = [
                ps_o.tile([128, H + 1], F32, tag=f"os{q}", name=f"osum{q}")
                for q in range(4)
            ]
            for jg in range((nj + 3) // 4):
                j0 = jg * 4
                jn = min(4, nj - j0)
                mg = mpool.tile([128, 4, QB], BF16, tag="mg", name="mg")
                nc.gpsimd.dma_start(
                    mg[:, 0:jn, :], m_r[:, j0 : j0 + jn, ts(i, QB)]
                )  # fp8 -> bf16 cast
                for jj in range(jn):
                    j = j0 + jj
                    sp = ps_big.tile([128, QB], F32, tag="big", name="sp")
                    nc.tensor.matmul(
                        sp[:], kt_s[:, ts(j, 128)], qt_s[:, ts(i, QB)],
                        start=True, stop=True,
                    )
                    pt = ppool.tile([128, QB], BF16, tag="pt", name="pt")
                    nc.scalar.activation(pt[:], sp[:], EXP, scale=0.125)
                    pm = ppool.tile([128, QB], BF16, tag="pm", name="pm")
                    nc.vector.tensor_mul(pm[:], pt[:], mg[:, jj, :])
                    for q in range(4):
                        nc.tensor.matmul(
                            osums[q][:],
                            pm[:, ts(q, 128)],
                            vext[:, j, :],
                            start=(j == 0),
                            stop=(j == nj - 1),
                        )
            for q in range(4):
                qc = 4 * i + q
                rec = opool.tile([128, 1], F32, tag="rec", name="rec")
                nc.vector.reciprocal(rec[:], osums[q][:, 64:65])
                ot = opool.tile([128, H], F32, tag="ot", name="ot")
                nc.vector.tensor_scalar_mul(ot[:], osums[q][:, 0:64], rec[:])
                nc.sync.dma_start(o_d[ts(qc, 128), :], ot[:])


def build_mask_t(random_cols: np.ndarray) -> np.ndarray:
    """Transposed bigbird mask [k_t, q_t] as fp8e4m3 (exact 0.0/1.0)."""
    import ml_dtypes

    rows = np.arange(T)[:, None]
    cols = np.arange(T)[None, :]
    causal = rows >= cols
    local = (cols >= rows - WINDOW + 1) & (cols <= rows)
    glob = (rows < N_GLOBAL) | (cols < N_GLOBAL)
    rand = np.zeros((T, T), dtype=bool)
    rand[np.arange(T)[:, None], random_cols.astype(np.int64)] = True
    mask = (local | glob | rand) & causal
    return np.ascontiguousarray(mask.T).astype(ml_dtypes.float8_e4m3)


def get_module():
    if "nc" not in _CACHE:
        _CACHE["nc"] = _build_module()
    return _CACHE["nc"]


def make_in_maps(x, random_cols, Wk, Wq, Wv):
    maskt = build_mask_t(np.asarray(random_cols))
    wq = np.ascontiguousarray(np.asarray(Wq, dtype=np.float32))
    wk = np.ascontiguousarray(np.asarray(Wk, dtype=np.float32))
    wv = np.ascontiguousarray(np.asarray(Wv, dtype=np.float32))
    x = np.asarray(x, dtype=np.float32)
    return [
        {
            "x": np.ascontiguousarray(x[b]),
            "wq": wq,
            "wk": wk,
            "wv": wv,
            "maskt": maskt,
        }
        for b in range(NCORES)
    ]


def kernel(x, random_cols, Wk, Wq, Wv, **_ignored):
    from concourse.bass_utils import run_bass_kernel_spmd

    nc = get_module()
    in_maps = make_in_maps(x, random_cols, Wk, Wq, Wv)
    res = run_bass_kernel_spmd(nc, in_maps, core_ids=list(range(NCORES)))
    out = np.stack([res.results[b]["out"] for b in range(NCORES)], axis=0)
    return out.astype(np.float32)


# revision 9
# speedup vs baseline: 1.1210x; 1.1210x over previous
"""BigBird sparse-attention head (B=8, T=2048, C=1024, H=64) on 8 TRN2 cores.

Sharding: data-parallel over batch — core b computes batch element b.
Per-core pipeline (all in one Tile kernel, scores kept in transposed
"sT" layout [k_t partitions, q_t free] so softmax normalization folds
into the P@V matmul via a ones-column on v):

  1. x [2048,1024] f32 --cast-DMA--> bf16, PE-transpose -> xT [1024,2048]
  2. Wq/Wk/Wv --cast--> bf16, PE-transpose; Wq/Wk packed as one lhsT
  3. qT,kT [64,2048] = WqkT.T @ xT   (PSUM f32, copied to bf16 SBUF)
     v    [2048,64] = xT.T @ WvT, stored as v_ext [128,16,65] with a
     ones column (col 64) so P@V also produces row sums
  4. for each q-block i (512 wide), k-chunk j (128): sT = kT_j.T @ qT_i;
     pT = exp(0.125*sT) (ACT, bf16); pT *= maskT tile (host-built fp8
     bigbird mask, cast to bf16 during DMA); out[128,65] += pT_q.T @ v_ext_j
  5. out[:, :64] * 1/out[:,64] -> DRAM

The bigbird mask (causal & (local|global|random)) depends only on
random_cols; it is built host-side once as fp8 (exact 0.0/1.0), shared
by all cores, and applied multiplicatively after exp. exp needs no
max-subtraction: scores are ~N(0,1) (|s|<~8) so exp is safe in f32/bf16.
"""

from contextlib import ExitStack

import numpy as np

B, T, C, H = 8, 2048, 1024, 64
WINDOW, N_GLOBAL, N_RANDOM = 64, 64, 64
NCORES = 8
TCH = T // 128   # 16 token chunks
CCH = C // 128   # 8 embed chunks
QB = 512         # q-block width
NQB = T // QB    # 4 q-blocks

_CACHE: dict = {}


def _build_module():
    import concourse.bacc as bacc
    import concourse.mybir as mybir
    import concourse.tile as tile

    F32 = mybir.dt.float32
    FP8 = mybir.dt.float8e4

    nc = bacc.Bacc(
        "TRN2", target_bir_lowering=False, debug=False, enable_asserts=False
    )
    x_d = nc.dram_tensor("x", [T, C], F32, kind="ExternalInput").ap()
    wq_d = nc.dram_tensor("wq", [H, C], F32, kind="ExternalInput").ap()
    wk_d = nc.dram_tensor("wk", [H, C], F32, kind="ExternalInput").ap()
    wv_d = nc.dram_tensor("wv", [H, C], F32, kind="ExternalInput").ap()
    m_d = nc.dram_tensor("maskt", [T, T], FP8, kind="ExternalInput").ap()
    o_d = nc.dram_tensor("out", [T, H], F32, kind="ExternalOutput").ap()

    with tile.TileContext(nc) as tc:
        _emit(tc, o_d, x_d, wq_d, wk_d, wv_d, m_d)
    nc.compile()
    return nc


def _emit(tc, o_d, x_d, wq_d, wk_d, wv_d, m_d):
    import concourse.mybir as mybir
    from concourse.bass import ts
    from concourse.masks import make_identity

    nc = tc.nc
    F32 = mybir.dt.float32
    BF16 = mybir.dt.bfloat16
    EXP = mybir.ActivationFunctionType.Exp

    with ExitStack() as ctx:
        sb = ctx.enter_context(tc.tile_pool(name="sb", bufs=3))
        persist = ctx.enter_context(tc.tile_pool(name="persist", bufs=1))
        ppool = ctx.enter_context(tc.tile_pool(name="ppool", bufs=3))
        mpool = ctx.enter_context(tc.tile_pool(name="mpool", bufs=2))
        opool = ctx.enter_context(tc.tile_pool(name="opool", bufs=2))
        ps_tp = ctx.enter_context(tc.tile_pool(name="ps_tp", bufs=2, space="PSUM"))
        ps_big = ctx.enter_context(tc.tile_pool(name="ps_big", bufs=2, space="PSUM"))
        ps_o = ctx.enter_context(tc.tile_pool(name="ps_o", bufs=1, space="PSUM"))

        ident = persist.tile([128, 128], BF16, tag="ident", name="ident")
        make_identity(nc, ident[:])
        identf = persist.tile([128, 128], F32, tag="identf", name="identf")
        make_identity(nc, identf[:])

        # ---- x: cast-load (batched) + PE-transpose into xT chunks -------
        xt = persist.tile([128, CCH, T], BF16, tag="xt", name="xt")
        x_r = x_d.rearrange("(n p) c -> p n c", p=128)
        for g in range(TCH // 4):
            xs4 = sb.tile([128, 4, C], BF16, tag="xstage", name="xs4")
            nc.gpsimd.dma_start(xs4[:], x_r[:, 4 * g : 4 * g + 4, :])  # f32->bf16
            for tt in range(4):
                t = 4 * g + tt
                for cg in range(CCH // 4):
                    pt4 = ps_tp.tile([128, 512], BF16, tag="tp", name="pt4")
                    for k in range(4):
                        c = cg * 4 + k
                        nc.tensor.transpose(
                            pt4[:, ts(k, 128)], xs4[:, tt, ts(c, 128)], ident[:]
                        )
                    dst = xt[:, cg * 4 : cg * 4 + 4, ts(t, 128)]
                    nc.vector.tensor_copy(
                        dst, pt4.rearrange("p (c f) -> p c f", f=128)
                    )

        # ---- weights: cast-load, PE-transpose, pack q|k ------------------
        wq_s = sb.tile([H, C], BF16, tag="wstage", name="wq_s")
        nc.gpsimd.dma_start(wq_s[:], wq_d[:])
        wk_s = sb.tile([H, C], BF16, tag="wstage", name="wk_s")
        nc.gpsimd.dma_start(wk_s[:], wk_d[:])
        wv_s = sb.tile([H, C], BF16, tag="wstage", name="wv_s")
        nc.gpsimd.dma_start(wv_s[:], wv_d[:])

        wqk = persist.tile([128, CCH, 128], BF16, tag="wqk", name="wqk")
        wvt = persist.tile([128, CCH, H], BF16, tag="wvt", name="wvt")
        for c in range(CCH):
            pw = ps_tp.tile([128, 512], BF16, tag="tp", name="pw")
            nc.tensor.transpose(pw[:, 0:64], wq_s[:, ts(c, 128)], ident[0:64, 0:64])
            nc.tensor.transpose(pw[:, 64:128], wk_s[:, ts(c, 128)], ident[0:64, 0:64])
            nc.tensor.transpose(pw[:, 128:192], wv_s[:, ts(c, 128)], ident[0:64, 0:64])
            nc.vector.tensor_copy(wqk[:, c, :], pw[:, 0:128])
            nc.vector.tensor_copy(wvt[:, c, :], pw[:, 128:192])

        # ---- projections qT, kT [64, 2048] -------------------------------
        qt_s = persist.tile([64, T], BF16, tag="qt", name="qt_s")
        kt_s = persist.tile([64, T], BF16, tag="kt", name="kt_s")
        for tb in range(NQB):
            pq = ps_big.tile([128, 512], F32, tag="big", name="pq")
            for c in range(CCH):
                nc.tensor.matmul(
                    pq[:],
                    wqk[:, c, :],
                    xt[:, c, ts(tb, 512)],
                    start=(c == 0),
                    stop=(c == CCH - 1),
                )
            nc.vector.tensor_copy(qt_s[:, ts(tb, 512)], pq[0:64, :])
            nc.vector.tensor_copy(kt_s[:, ts(tb, 512)], pq[64:128, :])

        # ---- v rows + ones column: v_ext [128, 16, 65] -------------------
        vext = persist.tile([128, TCH, H + 1], BF16, tag="vext", name="vext")
        for t in range(TCH):
            pv = ps_tp.tile([128, 512], F32, tag="tp", name="pv")
            for c in range(CCH):
                nc.tensor.matmul(
                    pv[:, 0:64],
                    xt[:, c, ts(t, 128)],
                    wvt[:, c, :],
                    start=(c == 0),
                    stop=(c == CCH - 1),
                )
            nc.vector.tensor_copy(vext[:, t, 0:64], pv[:, 0:64])
        nc.gpsimd.memset(vext[:, :, 64:65], 1.0)

        # ---- attention ---------------------------------------------------
        m_r = m_d.rearrange("(j p) q -> p j q", p=128)
        for i in range(NQB):
            nj = 4 * i + 4
            otp = ps_o.tile([H + 1, QB], F32, tag="ot", name="otp", bufs=2)
            for jg in range((nj + 3) // 4):
                j0 = jg * 4
                jn = min(4, nj - j0)
                mg = mpool.tile([128, 4, QB], BF16, tag="mg", name="mg")
                nc.gpsimd.dma_start(
                    mg[:, 0:jn, :], m_r[:, j0 : j0 + jn, ts(i, QB)]
                )  # fp8 -> bf16 cast
                for jj in range(jn):
                    j = j0 + jj
                    sp = ps_big.tile([128, QB], F32, tag="big", name="sp")
                    nc.tensor.matmul(
                        sp[:], kt_s[:, ts(j, 128)], qt_s[:, ts(i, QB)],
                        start=True, stop=True,
                    )
                    pt = ppool.tile([128, QB], BF16, tag="pt", name="pt")
                    nc.scalar.activation(pt[:], sp[:], EXP, scale=0.125)
                    pm = ppool.tile([128, QB], BF16, tag="pm", name="pm")
                    nc.vector.tensor_mul(pm[:], pt[:], mg[:, jj, :])
                    # out^T [65, 512] += v_ext_j.T @ p ; col 64 of v_ext is
                    # ones so row 64 accumulates the softmax denominators
                    nc.tensor.matmul(
                        otp[:], vext[:, j, :], pm[:],
                        start=(j == 0), stop=(j == nj - 1),
                    )
            ots = opool.tile([H + 1, QB], F32, tag="ots", name="ots")
            nc.vector.tensor_copy(ots[:], otp[:])
            for q in range(4):
                qc = 4 * i + q
                fin = ps_o.tile([128, H + 1], F32, tag="fin", name="fin", bufs=2)
                nc.tensor.transpose(
                    fin[:, 0 : H + 1], ots[:, ts(q, 128)], identf[0 : H + 1, 0 : H + 1]
                )
                rec = opool.tile([128, 1], F32, tag="rec", name="rec")
                nc.vector.reciprocal(rec[:], fin[:, 64:65])
                ot = opool.tile([128, H], F32, tag="ot", name="ot")
                nc.vector.tensor_scalar_mul(ot[:], fin[:, 0:64], rec[:])
                nc.sync.dma_start(o_d[ts(qc, 128), :], ot[:])


def build_mask_t(random_cols: np.ndarray) -> np.ndarray:
    """Transposed bigbird mask [k_t, q_t] as fp8e4m3 (exact 0.0/1.0)."""
    import ml_dtypes

    rows = np.arange(T)[:, None]
    cols = np.arange(T)[None, :]
    causal = rows >= cols
    local = (cols >= rows - WINDOW + 1) & (cols <= rows)
    glob = (rows < N_GLOBAL) | (cols < N_GLOBAL)
    rand = np.zeros((T, T), dtype=bool)
    rand[np.arange(T)[:, None], random_cols.astype(np.int64)] = True
    mask = (local | glob | rand) & causal
    return np.ascontiguousarray(mask.T).astype(ml_dtypes.float8_e4m3)


def get_module():
    if "nc" not in _CACHE:
        _CACHE["nc"] = _build_module()
    return _CACHE["nc"]


def make_in_maps(x, random_cols, Wk, Wq, Wv):
    maskt = build_mask_t(np.asarray(random_cols))
    wq = np.ascontiguousarray(np.asarray(Wq, dtype=np.float32))
    wk = np.ascontiguousarray(np.asarray(Wk, dtype=np.float32))
    wv = np.ascontiguousarray(np.asarray(Wv, dtype=np.float32))
    x = np.asarray(x, dtype=np.float32)
    return [
        {
            "x": np.ascontiguousarray(x[b]),
            "wq": wq,
            "wk": wk,
            "wv": wv,
            "maskt": maskt,
        }
        for b in range(NCORES)
    ]


def kernel(x, random_cols, Wk, Wq, Wv, **_ignored):
    from concourse.bass_utils import run_bass_kernel_spmd

    nc = get_module()
    in_maps = make_in_maps(x, random_cols, Wk, Wq, Wv)
    res = run_bass_kernel_spmd(nc, in_maps, core_ids=list(range(NCORES)))
    out = np.stack([res.results[b]["out"] for b in range(NCORES)], axis=0)
    return out.astype(np.float32)
